# revision 1
# baseline (speedup 1.0000x reference)
"""Causal self-attention (B=2, T=2048, D=1024, H=16) on 8 TRN2 NeuronCores.

Sharding: data-parallel over batch (cores 0-3 -> batch 0, cores 4-7 -> batch 1),
tensor-parallel over heads (4 heads / 256 output dims per core). Each core
computes q/k/v projections for its heads, causal flash-style attention, and a
partial output projection (contraction over its 256 dims of Wo). The host sums
the 4 partials per batch and adds bo.

All matmuls run in float32r (full fp32 storage, ~tf32-class matmul precision,
4x the throughput of strict fp32 on the PE).
"""
import sys

sys.path.insert(0, '/opt/trn_rl_repo')

import numpy as np

import concourse.bass as bass  # noqa: F401  (import keeps bass registered)
import concourse.mybir as mybir
import concourse.tile as tile
from concourse import bacc
from concourse.bass_utils import run_bass_kernel_spmd

F32 = mybir.dt.float32
F32R = mybir.dt.float32r
BF16 = mybir.dt.bfloat16
AF = mybir.ActivationFunctionType

B, T, D, H, HD = 2, 2048, 1024, 16, 64
NCORES = 8
E = 256          # output dims per core (4 heads x 64)
DM = 8           # d_model chunks of 128
DMA_ = 9         # augmented chunks (ones row for v bias)
DAUG = DMA_ * 128
TQ = 512
NTQ = T // TQ    # 4
TKT = 128
NTKT = T // TKT  # 16

_CACHE = {}


def _build():
    nc = bacc.Bacc("TRN2", target_bir_lowering=False, debug=False)

    xT = nc.dram_tensor("xT", [D, T], F32R, kind="ExternalInput")
    wq = nc.dram_tensor("wq", [D, E], F32R, kind="ExternalInput")
    wk = nc.dram_tensor("wk", [D, E], F32R, kind="ExternalInput")
    wv = nc.dram_tensor("wv", [D, E], F32R, kind="ExternalInput")
    wo = nc.dram_tensor("wo", [E, D], F32R, kind="ExternalInput")
    bq_d = nc.dram_tensor("bq", [E, 1], F32, kind="ExternalInput")
    bk_d = nc.dram_tensor("bk", [E, 1], F32, kind="ExternalInput")
    bvb_d = nc.dram_tensor("bvb", [128, E], F32R, kind="ExternalInput")
    onesc_d = nc.dram_tensor("onesc", [128, 4], F32R, kind="ExternalInput")
    onesr_d = nc.dram_tensor("onesr", [33, HD], F32R, kind="ExternalInput")
    outT = nc.dram_tensor("outT", [D, T], F32, kind="ExternalOutput")

    with tile.TileContext(nc) as tc, nc.allow_low_precision(reason="fp32r/bf16 attn"):
        with (
            tc.tile_pool(name="persist", bufs=1) as pp,
            tc.tile_pool(name="xw", bufs=1) as xw,
            tc.tile_pool(name="work", bufs=5) as wk_pool,
            tc.tile_pool(name="ostage", bufs=3) as op_pool,
            tc.tile_pool(name="small", bufs=1) as sm,
            tc.tile_pool(name="psum", bufs=2, space="PSUM") as ps,
        ):
            # ---- input DMAs, ordered so the first projection chains start asap
            xT_sb = [xw.tile([128, T], F32R, tag=f"x{c}", name=f"x{c}")
                     for c in range(DM)]
            wq_sb = [xw.tile([128, E], F32R, tag=f"wq{c}", name=f"wq{c}")
                     for c in range(DM)]
            wk_sb = [xw.tile([128, E], F32R, tag=f"wk{c}", name=f"wk{c}")
                     for c in range(DM)]
            wv_sb = [xw.tile([128, E], F32R, tag=f"wv{c}", name=f"wv{c}")
                     for c in range(DM)]
            wo_sb = [pp.tile([128, D], F32R, tag=f"wo{d2}", name=f"wo{d2}")
                     for d2 in range(2)]
            nc.sync.dma_start(
                out=xT_sb[0][:, 0:TQ], in_=xT[0:128, 0:TQ])
            nc.sync.dma_start(out=wq_sb[0][:], in_=wq[0:128, :])
            nc.sync.dma_start(out=wk_sb[0][:], in_=wk[0:128, :])
            bvb = pp.tile([128, E], F32R, tag="bvb")
            nc.sync.dma_start(out=bvb[:], in_=bvb_d[:, :])
            bq_sb, bk_sb = [], []
            for e2 in range(2):
                t_ = pp.tile([128, 1], F32, tag=f"bq{e2}")
                nc.sync.dma_start(out=t_[:], in_=bq_d[e2 * 128:(e2 + 1) * 128, :])
                bq_sb.append(t_)
                t_ = pp.tile([128, 1], F32, tag=f"bk{e2}")
                nc.sync.dma_start(out=t_[:], in_=bk_d[e2 * 128:(e2 + 1) * 128, :])
                bk_sb.append(t_)
            onesc = pp.tile([128, 4], F32R, tag="onesc")
            nc.sync.dma_start(out=onesc[:], in_=onesc_d[:, :])
            onesr = pp.tile([33, HD], F32R, tag="onesr")
            nc.sync.dma_start(out=onesr[:], in_=onesr_d[:, :])
            for c in range(DM):
                if c > 0:
                    nc.sync.dma_start(
                        out=xT_sb[c][:, 0:TQ], in_=xT[c * 128:(c + 1) * 128, 0:TQ])
                    nc.sync.dma_start(out=wq_sb[c][:], in_=wq[c * 128:(c + 1) * 128, :])
                    nc.sync.dma_start(out=wk_sb[c][:], in_=wk[c * 128:(c + 1) * 128, :])
                nc.sync.dma_start(out=wv_sb[c][:], in_=wv[c * 128:(c + 1) * 128, :])
            for c in range(DM):
                nc.sync.dma_start(
                    out=xT_sb[c][:, TQ:2 * TQ], in_=xT[c * 128:(c + 1) * 128, TQ:2 * TQ])
            for c in range(DM):
                nc.sync.dma_start(
                    out=xT_sb[c][:, 2 * TQ:3 * TQ],
                    in_=xT[c * 128:(c + 1) * 128, 2 * TQ:3 * TQ])
            for d2 in range(2):
                nc.sync.dma_start(out=wo_sb[d2][:], in_=wo[d2 * 128:(d2 + 1) * 128, :])
            for c in range(DM):
                nc.sync.dma_start(
                    out=xT_sb[c][:, 3 * TQ:4 * TQ],
                    in_=xT[c * 128:(c + 1) * 128, 3 * TQ:4 * TQ])

            qT_sb = [pp.tile([128, T], F32R, tag=f"qT{i}", name=f"qT{i}") for i in range(2)]
            kT_sb = [pp.tile([128, T], F32R, tag=f"kT{i}", name=f"kT{i}") for i in range(2)]
            v_sb = [pp.tile([128, 4, HD + 1], F32R, tag=f"v{t}", name=f"v{t}")
                    for t in range(NTKT)]
            yT_sb = [pp.tile([128, T], F32R, tag=f"yT{i}", name=f"yT{i}") for i in range(2)]

            def project_qk(tq):
                for (w_sb, b_sb, dst) in ((wq_sb, bq_sb, qT_sb), (wk_sb, bk_sb, kT_sb)):
                    for e2 in range(2):
                        pt = ps.tile([128, 1024], F32, tag="S",
                                     name=f"ppqk_{tq}_{e2}")
                        for c in range(DM):
                            nc.tensor.matmul(
                                pt[:, 0:TQ],
                                w_sb[c][:, e2 * 128:(e2 + 1) * 128],
                                xT_sb[c][:, tq * TQ:(tq + 1) * TQ],
                                start=(c == 0), stop=(c == DM - 1))
                        nc.vector.tensor_scalar_add(
                            out=dst[e2][:, tq * TQ:(tq + 1) * TQ],
                            in0=pt[:, 0:TQ], scalar1=b_sb[e2][:])

            def project_v(t):
                pt = ps.tile([128, E], F32, tag="y", name=f"ppv_{t}")
                for c in range(DM):
                    nc.tensor.matmul(
                        pt[:],
                        xT_sb[c][:, t * 128:(t + 1) * 128],
                        wv_sb[c][:],
                        start=(c == 0), stop=(c == DM - 1))
                nc.vector.tensor_add(
                    out=v_sb[t][:, :, 0:HD],
                    in0=pt[:].rearrange("p (h d) -> p h d", h=4),
                    in1=bvb[:].rearrange("p (h d) -> p h d", h=4))
                nc.vector.tensor_copy(
                    out=v_sb[t][:, :, HD:HD + 1],
                    in_=onesc[:].rearrange("p (h o) -> p h o", o=1))

            def out_proj_chain(tq_o, e8):
                pt = ps.tile([128, TQ], F32, tag="b", name=f"poc_{tq_o}_{e8}")
                for d2 in range(2):
                    nc.tensor.matmul(
                        pt[:, 0:TQ],
                        wo_sb[d2][:, e8 * 128:(e8 + 1) * 128],
                        yT_sb[d2][:, tq_o * TQ:(tq_o + 1) * TQ],
                        start=(d2 == 0), stop=(d2 == 1))
                ot = op_pool.tile([128, TQ], F32, tag="ostage", name=f"oto_{tq_o}_{e8}")
                nc.vector.tensor_copy(out=ot[:], in_=pt[:])
                nc.sync.dma_start(
                    out=outT[e8 * 128:(e8 + 1) * 128, tq_o * TQ:(tq_o + 1) * TQ],
                    in_=ot[:])

            oproj_queue = []

            def attention(tq):
                ntk = 4 * (tq + 1)
                for pr in range(2):
                    kt = kT_sb[pr]
                    qt = qT_sb[pr]
                    py_a = ps.tile([HD + 1, TQ], F32, tag="y", name=f"pya_{tq}_{pr}")
                    py_b = ps.tile([HD + 1, TQ], F32, tag="y", name=f"pyb_{tq}_{pr}")

                    def s_stage(tk):
                        # diag tiles only need columns >= 128*o (o = tk - 4*tq)
                        o = tk - 4 * tq
                        c0 = 128 * o if o > 0 else 0
                        n = TQ - c0
                        ps_s = ps.tile([128, 1024], F32, tag="S",
                                       name=f"ps_s_{tq}_{pr}_{tk}")
                        q0 = tq * TQ + c0
                        nc.tensor.matmul(
                            ps_s[:, c0:TQ],
                            kt[0:64, tk * 128:(tk + 1) * 128],
                            qt[0:64, q0:(tq + 1) * TQ],
                            start=True, stop=True)
                        nc.tensor.matmul(
                            ps_s[:, TQ + c0:2 * TQ],
                            kt[64:128, tk * 128:(tk + 1) * 128],
                            qt[64:128, q0:(tq + 1) * TQ],
                            start=True, stop=True)
                        es = wk_pool.tile([128, 1024], F32R, tag="expS",
                                          name=f"es_{tq}_{pr}_{tk}")
                        if c0 == 0:
                            nc.scalar.activation(es[:], ps_s[:], AF.Exp, scale=0.125)
                        elif c0 <= 256:
                            # one contiguous op; the [TQ, TQ+c0) junk span is never read
                            nc.scalar.activation(
                                es[:, c0:2 * TQ], ps_s[:, c0:2 * TQ],
                                AF.Exp, scale=0.125)
                        else:
                            for j in range(2):
                                nc.scalar.activation(
                                    es[:, j * TQ + c0:(j + 1) * TQ],
                                    ps_s[:, j * TQ + c0:(j + 1) * TQ],
                                    AF.Exp, scale=0.125)
                        if o >= 0:
                            em = wk_pool.tile([128, 1024], F32R, tag="expS",
                                              name=f"em_{tq}_{pr}_{tk}")
                            for j in range(2):
                                nc.gpsimd.affine_select(
                                    out=em[:, j * TQ + c0:(j + 1) * TQ],
                                    in_=es[:, j * TQ + c0:(j + 1) * TQ],
                                    compare_op=mybir.AluOpType.is_ge,
                                    fill=0.0,
                                    base=0,
                                    pattern=[[1, n]],
                                    channel_multiplier=-1)
                            es = em
                        return es, c0

                    def y_stage(tk, es, c0):
                        nc.tensor.matmul(
                            py_a[:, c0:TQ], v_sb[tk][:, 2 * pr, :],
                            es[:, c0:TQ],
                            start=(tk == 0), stop=(tk == ntk - 1))
                        nc.tensor.matmul(
                            py_b[:, c0:TQ], v_sb[tk][:, 2 * pr + 1, :],
                            es[:, TQ + c0:2 * TQ],
                            start=(tk == 0), stop=(tk == ntk - 1))

                    prev = s_stage(0)
                    for tk in range(1, ntk):
                        cur = s_stage(tk)
                        y_stage(tk - 1, *prev)
                        prev = cur
                    y_stage(ntk - 1, *prev)

                    dn = sm.tile([33, TQ], F32, tag="dn")
                    nc.vector.tensor_copy(out=dn[0:1, :], in_=py_a[HD:HD + 1, :])
                    nc.vector.tensor_copy(out=dn[32:33, :], in_=py_b[HD:HD + 1, :])
                    scr = sm.tile([33, TQ], F32, tag="scr")
                    rc32 = sm.tile([33, TQ], F32, tag="rc32")
                    nc.vector.reciprocal_approx_accurate(
                        out=rc32[:, :], in_=dn[:, :], scratch=scr[:, :])
                    rc = sm.tile([33, TQ], F32R, tag="rc")
                    nc.vector.tensor_copy(out=rc[:, :], in_=rc32[:, :])
                    for (i, py) in ((0, py_a), (1, py_b)):
                        pb = ps.tile([HD, TQ], F32, tag="b", name=f"pb_{tq}_{pr}_{i}")
                        nc.tensor.matmul(pb[:], onesr[32 * i:32 * i + 1, :],
                                         rc[32 * i:32 * i + 1, :],
                                         start=True, stop=True)
                        bc = sm.tile([HD, TQ], F32, tag="bc")
                        nc.vector.tensor_copy(out=bc[:], in_=pb[:])
                        row0 = i * 64
                        nc.vector.tensor_mul(
                            out=yT_sb[pr][row0:row0 + 64, tq * TQ:(tq + 1) * TQ],
                            in0=py[0:HD, :], in1=bc[:])

            # ---- interleaved emission: per tq block, project then attend ----
            for tq in range(NTQ):
                project_qk(tq)
                for t in range(4 * tq, 4 * tq + 4):
                    project_v(t)
                if tq > 0:
                    oproj_queue.extend((tq - 1, e8) for e8 in range(8))
                attention(tq)
            while oproj_queue:
                out_proj_chain(*oproj_queue.pop(0))
            for e8 in range(8):
                out_proj_chain(NTQ - 1, e8)

    nc.compile()
    return nc


def _get_nc():
    if 'nc' not in _CACHE:
        _CACHE['nc'] = _build()
    return _CACHE['nc']


def _make_in_maps(x, Wq, bq, Wk, bk, Wv, bv, Wo, bo):
    x = np.asarray(x, dtype=np.float32)
    Wq = np.asarray(Wq, dtype=np.float32)
    Wk = np.asarray(Wk, dtype=np.float32)
    Wv = np.asarray(Wv, dtype=np.float32)
    Wo = np.asarray(Wo, dtype=np.float32)
    bq = np.asarray(bq, dtype=np.float32)
    bk = np.asarray(bk, dtype=np.float32)
    bv = np.asarray(bv, dtype=np.float32)

    import ml_dtypes
    onesc = np.ones((128, 4), dtype=np.float32)
    onesr = np.ones((33, HD), dtype=np.float32)

    in_maps = []
    for c in range(NCORES):
        b, g = divmod(c, 4)
        hs = slice(g * E, (g + 1) * E)
        in_maps.append({
            "xT": np.ascontiguousarray(x[b].T),
            "wq": np.ascontiguousarray(Wq[hs].T),
            "wk": np.ascontiguousarray(Wk[hs].T),
            "wv": np.ascontiguousarray(Wv[hs].T),
            "wo": np.ascontiguousarray(Wo[:, hs].T),
            "bq": np.ascontiguousarray(bq[hs].reshape(E, 1)),
            "bk": np.ascontiguousarray(bk[hs].reshape(E, 1)),
            "bvb": np.broadcast_to(bv[hs], (128, E)).copy(),
            "onesc": onesc,
            "onesr": onesr,
        })
    return in_maps


def kernel(x, Wq, bq, Wk, bk, Wv, bv, Wo, bo, _run_kwargs=None):
    nc = _get_nc()
    in_maps = _make_in_maps(x, Wq, bq, Wk, bk, Wv, bv, Wo, bo)
    last_err = None
    for _attempt in range(3):
        try:
            res = run_bass_kernel_spmd(nc, in_maps, core_ids=list(range(NCORES)),
                                       **(_run_kwargs or {}))
            break
        except Exception as e:  # transient NRT/device hiccups: retry
            last_err = e
            import time as _time
            _time.sleep(2.0)
    else:
        raise last_err
    bo = np.asarray(bo, dtype=np.float32)
    out = np.empty((B, T, D), dtype=np.float32)
    for b in range(B):
        acc = res.results[4 * b]["outT"].copy()
        for g in range(1, 4):
            acc += res.results[4 * b + g]["outT"]
        out[b] = acc.T + bo
    if _run_kwargs:
        _CACHE['last_results'] = res
    return out



# revision 4
# speedup vs baseline: 1.3786x; 1.3786x over previous
"""Causal self-attention (B=2, T=2048, D=1024, H=16) on 8 TRN2 NeuronCores.

Sharding: data-parallel over batch (cores 0-3 -> batch 0, cores 4-7 -> batch 1),
tensor-parallel over heads (4 heads / 256 output dims per core). Each core
computes q/k/v projections for its heads, causal flash-style attention, and a
partial output projection (contraction over its 256 dims of Wo). The host sums
the 4 partials per batch and adds bo.

All matmuls run in bf16 (fp32 PSUM accumulation). Projection / output-
projection matmuls are woven one-at-a-time into the attention tile loop so the
PE never idles waiting for the scalar engine's exp.
"""
import sys

sys.path.insert(0, '/opt/trn_rl_repo')

import numpy as np

import concourse.bass as bass  # noqa: F401  (import keeps bass registered)
import concourse.mybir as mybir
import concourse.tile as tile
from concourse import bacc
from concourse.bass_utils import run_bass_kernel_spmd

F32 = mybir.dt.float32
BF16 = mybir.dt.bfloat16
AF = mybir.ActivationFunctionType

B, T, D, H, HD = 2, 2048, 1024, 16, 64
NCORES = 8
E = 256          # output dims per core (4 heads x 64)
DM = 8           # d_model chunks of 128
TQ = 512
NTQ = T // TQ    # 4
TKT = 128
NTKT = T // TKT  # 16

_CACHE = {}


def _build():
    nc = bacc.Bacc("TRN2", target_bir_lowering=False, debug=False)

    xT = nc.dram_tensor("xT", [D, T], BF16, kind="ExternalInput")
    wq = nc.dram_tensor("wq", [D, E], BF16, kind="ExternalInput")
    wk = nc.dram_tensor("wk", [D, E], BF16, kind="ExternalInput")
    wv = nc.dram_tensor("wv", [D, E], BF16, kind="ExternalInput")
    wo = nc.dram_tensor("wo", [E, D], BF16, kind="ExternalInput")
    bq_d = nc.dram_tensor("bq", [E, 1], F32, kind="ExternalInput")
    bk_d = nc.dram_tensor("bk", [E, 1], F32, kind="ExternalInput")
    bvb_d = nc.dram_tensor("bvb", [128, E], F32, kind="ExternalInput")
    onesr_d = nc.dram_tensor("onesr", [33, HD], BF16, kind="ExternalInput")
    outT = nc.dram_tensor("outT", [D, T], BF16, kind="ExternalOutput")

    with tile.TileContext(nc) as tc, nc.allow_low_precision(reason="bf16 attn"):
        with (
            tc.tile_pool(name="persist", bufs=1) as pp,
            tc.tile_pool(name="xw", bufs=1) as xw,
            tc.tile_pool(name="work", bufs=6) as wk_pool,
            tc.tile_pool(name="ostage", bufs=3) as op_pool,
            tc.tile_pool(name="small", bufs=2) as sm,
            tc.tile_pool(name="psS", bufs=2, space="PSUM") as ps_s,
            tc.tile_pool(name="psY", bufs=1, space="PSUM") as ps_y,
            tc.tile_pool(name="psP", bufs=2, space="PSUM") as ps_p,
        ):
            # ---- input DMAs, ordered so the first projection chains start asap
            xT_sb = [xw.tile([128, T], BF16, tag=f"x{c}", name=f"x{c}")
                     for c in range(DM)]
            wq_sb = [xw.tile([128, E], BF16, tag=f"wq{c}", name=f"wq{c}")
                     for c in range(DM)]
            wk_sb = [xw.tile([128, E], BF16, tag=f"wk{c}", name=f"wk{c}")
                     for c in range(DM)]
            wv_sb = [xw.tile([128, E], BF16, tag=f"wv{c}", name=f"wv{c}")
                     for c in range(DM)]
            wo_sb = [pp.tile([128, D], BF16, tag=f"wo{d2}", name=f"wo{d2}")
                     for d2 in range(2)]
            for c in range(DM):
                nc.sync.dma_start(out=wq_sb[c][:], in_=wq[c * 128:(c + 1) * 128, :])
                nc.sync.dma_start(
                    out=xT_sb[c][:, 0:TQ], in_=xT[c * 128:(c + 1) * 128, 0:TQ])
            bvb = pp.tile([128, E], F32, tag="bvb")
            nc.sync.dma_start(out=bvb[:], in_=bvb_d[:, :])
            bq_sb, bk_sb = [], []
            for e2 in range(2):
                t_ = pp.tile([128, 1], F32, tag=f"bq{e2}")
                nc.sync.dma_start(out=t_[:], in_=bq_d[e2 * 128:(e2 + 1) * 128, :])
                bq_sb.append(t_)
                t_ = pp.tile([128, 1], F32, tag=f"bk{e2}")
                nc.sync.dma_start(out=t_[:], in_=bk_d[e2 * 128:(e2 + 1) * 128, :])
                bk_sb.append(t_)
            onesr = pp.tile([33, HD], BF16, tag="onesr")
            nc.sync.dma_start(out=onesr[:], in_=onesr_d[:, :])
            for c in range(DM):
                nc.sync.dma_start(out=wk_sb[c][:], in_=wk[c * 128:(c + 1) * 128, :])
                nc.sync.dma_start(out=wv_sb[c][:], in_=wv[c * 128:(c + 1) * 128, :])
            for c in range(DM):
                nc.sync.dma_start(
                    out=xT_sb[c][:, TQ:2 * TQ], in_=xT[c * 128:(c + 1) * 128, TQ:2 * TQ])
            for c in range(DM):
                nc.sync.dma_start(
                    out=xT_sb[c][:, 2 * TQ:3 * TQ],
                    in_=xT[c * 128:(c + 1) * 128, 2 * TQ:3 * TQ])
            for d2 in range(2):
                nc.sync.dma_start(out=wo_sb[d2][:], in_=wo[d2 * 128:(d2 + 1) * 128, :])
            for c in range(DM):
                nc.sync.dma_start(
                    out=xT_sb[c][:, 3 * TQ:4 * TQ],
                    in_=xT[c * 128:(c + 1) * 128, 3 * TQ:4 * TQ])

            qT_sb = [pp.tile([128, T], BF16, tag=f"qT{i}", name=f"qT{i}") for i in range(2)]
            kT_sb = [pp.tile([128, T], BF16, tag=f"kT{i}", name=f"kT{i}") for i in range(2)]
            v_sb = [pp.tile([128, 4, HD + 1], BF16, tag=f"v{t}", name=f"v{t}")
                    for t in range(NTKT)]
            yT_sb = [pp.tile([128, T], BF16, tag=f"yT{i}", name=f"yT{i}") for i in range(2)]

            # constant ones column of v (denominator trick), set once
            for t in range(NTKT):
                nc.gpsimd.memset(v_sb[t][:, :, HD:HD + 1], 1.0)

            def gen_qk(tq):
                """Yields once per matmul; q/k projection for query window tq."""
                for wi, (w_sb, b_sb, dst) in enumerate(
                        ((wq_sb, bq_sb, qT_sb), (wk_sb, bk_sb, kT_sb))):
                    for e2 in range(2):
                        pt = ps_p.tile([128, TQ], F32, tag="p",
                                       name=f"ppqk_{tq}_{wi}_{e2}")
                        for c in range(DM):
                            nc.tensor.matmul(
                                pt[:],
                                w_sb[c][:, e2 * 128:(e2 + 1) * 128],
                                xT_sb[c][:, tq * TQ:(tq + 1) * TQ],
                                start=(c == 0), stop=(c == DM - 1))
                            if c < DM - 1:
                                yield
                        nc.vector.tensor_scalar_add(
                            out=dst[e2][:, tq * TQ:(tq + 1) * TQ],
                            in0=pt[:], scalar1=b_sb[e2][:])
                        yield

            def gen_v(trange):
                """Yields once per matmul; v projection for 128-token tiles."""
                for t in trange:
                    pt = ps_p.tile([128, E], F32, tag="p", name=f"ppv_{t}")
                    for c in range(DM):
                        nc.tensor.matmul(
                            pt[:],
                            xT_sb[c][:, t * 128:(t + 1) * 128],
                            wv_sb[c][:],
                            start=(c == 0), stop=(c == DM - 1))
                        if c < DM - 1:
                            yield
                    nc.vector.tensor_add(
                        out=v_sb[t][:, :, 0:HD],
                        in0=pt[:].rearrange("p (h d) -> p h d", h=4),
                        in1=bvb[:].rearrange("p (h d) -> p h d", h=4))
                    yield

            def gen_oproj(tq_o):
                """Yields once per matmul; partial out-proj for window tq_o."""
                for e8 in range(8):
                    pt = ps_p.tile([128, TQ], F32, tag="p", name=f"poc_{tq_o}_{e8}")
                    for d2 in range(2):
                        nc.tensor.matmul(
                            pt[:],
                            wo_sb[d2][:, e8 * 128:(e8 + 1) * 128],
                            yT_sb[d2][:, tq_o * TQ:(tq_o + 1) * TQ],
                            start=(d2 == 0), stop=(d2 == 1))
                        if d2 == 0:
                            yield
                    ot = op_pool.tile([128, TQ], BF16, tag="ostage",
                                      name=f"oto_{tq_o}_{e8}")
                    nc.vector.tensor_copy(out=ot[:], in_=pt[:])
                    nc.sync.dma_start(
                        out=outT[e8 * 128:(e8 + 1) * 128, tq_o * TQ:(tq_o + 1) * TQ],
                        in_=ot[:])
                    yield

            def s_stage(tq, pr, tk):
                """S matmuls + exp (+ causal mask for diagonal tiles)."""
                kt = kT_sb[pr]
                qt = qT_sb[pr]
                # diag tiles only need columns >= 128*o (o = tk - 4*tq)
                o = tk - 4 * tq
                c0 = 128 * o if o > 0 else 0
                n = TQ - c0
                ps_t = ps_s.tile([128, 1024], F32, tag="S",
                                 name=f"ps_s_{tq}_{pr}_{tk}")
                q0 = tq * TQ + c0
                nc.tensor.matmul(
                    ps_t[:, c0:TQ],
                    kt[0:64, tk * 128:(tk + 1) * 128],
                    qt[0:64, q0:(tq + 1) * TQ],
                    start=True, stop=True)
                nc.tensor.matmul(
                    ps_t[:, TQ + c0:2 * TQ],
                    kt[64:128, tk * 128:(tk + 1) * 128],
                    qt[64:128, q0:(tq + 1) * TQ],
                    start=True, stop=True)
                es = wk_pool.tile([128, 1024], BF16, tag="expS",
                                  name=f"es_{tq}_{pr}_{tk}")
                if c0 == 0:
                    nc.scalar.activation(es[:], ps_t[:], AF.Exp, scale=0.125)
                elif c0 <= 256:
                    # one contiguous op; the [TQ, TQ+c0) junk span is never read
                    nc.scalar.activation(
                        es[:, c0:2 * TQ], ps_t[:, c0:2 * TQ],
                        AF.Exp, scale=0.125)
                else:
                    for j in range(2):
                        nc.scalar.activation(
                            es[:, j * TQ + c0:(j + 1) * TQ],
                            ps_t[:, j * TQ + c0:(j + 1) * TQ],
                            AF.Exp, scale=0.125)
                if o >= 0:
                    em = wk_pool.tile([128, 1024], BF16, tag="expS",
                                      name=f"em_{tq}_{pr}_{tk}")
                    for j in range(2):
                        nc.gpsimd.affine_select(
                            out=em[:, j * TQ + c0:(j + 1) * TQ],
                            in_=es[:, j * TQ + c0:(j + 1) * TQ],
                            compare_op=mybir.AluOpType.is_ge,
                            fill=0.0,
                            base=0,
                            pattern=[[1, n]],
                            channel_multiplier=-1)
                    es = em
                return es, c0

            def y_stage(tq, pr, tk, py, es, c0, ntk):
                nc.tensor.matmul(
                    py[:, c0:TQ], v_sb[tk][:, 2 * pr, :],
                    es[:, c0:TQ],
                    start=(tk == 0), stop=(tk == ntk - 1))
                nc.tensor.matmul(
                    py[:, TQ + c0:2 * TQ], v_sb[tk][:, 2 * pr + 1, :],
                    es[:, TQ + c0:2 * TQ],
                    start=(tk == 0), stop=(tk == ntk - 1))

            def normalize(tq, pr, py):
                dn = sm.tile([33, TQ], F32, tag="dn", name=f"dn_{tq}_{pr}")
                nc.vector.tensor_copy(out=dn[0:1, :], in_=py[HD:HD + 1, 0:TQ])
                nc.vector.tensor_copy(out=dn[32:33, :], in_=py[HD:HD + 1, TQ:2 * TQ])
                scr = sm.tile([33, TQ], F32, tag="scr", name=f"scr_{tq}_{pr}")
                rc32 = sm.tile([33, TQ], F32, tag="rc32", name=f"rc32_{tq}_{pr}")
                nc.vector.reciprocal_approx_accurate(
                    out=rc32[:, :], in_=dn[:, :], scratch=scr[:, :])
                rc = sm.tile([33, TQ], BF16, tag="rc", name=f"rc_{tq}_{pr}")
                nc.vector.tensor_copy(out=rc[:, :], in_=rc32[:, :])
                pb = ps_p.tile([128, TQ], F32, tag="p", name=f"pb_{tq}_{pr}")
                for i in range(2):
                    nc.tensor.matmul(pb[i * 64:(i + 1) * 64, :],
                                     onesr[32 * i:32 * i + 1, :],
                                     rc[32 * i:32 * i + 1, :],
                                     start=True, stop=True)
                bc = sm.tile([128, TQ], BF16, tag="bc", name=f"bc_{tq}_{pr}")
                nc.vector.tensor_copy(out=bc[:], in_=pb[:])
                for i in range(2):
                    row0 = i * 64
                    nc.vector.tensor_mul(
                        out=yT_sb[pr][row0:row0 + 64, tq * TQ:(tq + 1) * TQ],
                        in0=py[0:HD, i * TQ:(i + 1) * TQ],
                        in1=bc[row0:row0 + 64, :])

            def drain(g, k=None):
                if g is None:
                    return
                if k is None:
                    for _ in g:
                        pass
                else:
                    for _ in range(k):
                        if next(g, StopIteration) is StopIteration:
                            break

            # ---- schedule ----
            drain(gen_qk(0))
            drain(gen_v(range(0, 4)))
            import itertools
            for tq in range(NTQ):
                ntk = 4 * (tq + 1)
                fillers = []
                if tq < NTQ - 1:
                    fillers.append(gen_qk(tq + 1))
                    fillers.append(gen_v(range(4 * (tq + 1), 4 * (tq + 1) + 4)))
                if tq > 0:
                    fillers.append(gen_oproj(tq - 1))
                fg = itertools.chain(*fillers)
                for pr in range(2):
                    py = ps_y.tile([65, 1024], F32, tag="y", name=f"py_{tq}_{pr}")
                    prev = None
                    for tk in range(ntk):
                        cur = s_stage(tq, pr, tk)
                        if prev is not None:
                            y_stage(tq, pr, tk - 1, py, *prev, ntk)
                        prev = cur
                        drain(fg, 1)
                    y_stage(tq, pr, ntk - 1, py, *prev, ntk)
                    normalize(tq, pr, py)
                    drain(fg, 12)
                drain(fg)
            drain(gen_oproj(NTQ - 1))

    nc.compile()
    return nc


def _get_nc():
    if 'nc' not in _CACHE:
        _CACHE['nc'] = _build()
    return _CACHE['nc']


def _make_in_maps(x, Wq, bq, Wk, bk, Wv, bv, Wo, bo):
    import ml_dtypes
    BF = ml_dtypes.bfloat16
    x = np.asarray(x, dtype=np.float32)
    Wq = np.asarray(Wq, dtype=np.float32)
    Wk = np.asarray(Wk, dtype=np.float32)
    Wv = np.asarray(Wv, dtype=np.float32)
    Wo = np.asarray(Wo, dtype=np.float32)
    bq = np.asarray(bq, dtype=np.float32)
    bk = np.asarray(bk, dtype=np.float32)
    bv = np.asarray(bv, dtype=np.float32)

    onesr = np.ones((33, HD), dtype=BF)

    in_maps = []
    for c in range(NCORES):
        b, g = divmod(c, 4)
        hs = slice(g * E, (g + 1) * E)
        in_maps.append({
            "xT": np.ascontiguousarray(x[b].T.astype(BF)),
            "wq": np.ascontiguousarray(Wq[hs].T.astype(BF)),
            "wk": np.ascontiguousarray(Wk[hs].T.astype(BF)),
            "wv": np.ascontiguousarray(Wv[hs].T.astype(BF)),
            "wo": np.ascontiguousarray(Wo[:, hs].T.astype(BF)),
            "bq": np.ascontiguousarray(bq[hs].reshape(E, 1)),
            "bk": np.ascontiguousarray(bk[hs].reshape(E, 1)),
            "bvb": np.broadcast_to(bv[hs], (128, E)).copy(),
            "onesr": onesr,
        })
    return in_maps


def kernel(x, Wq, bq, Wk, bk, Wv, bv, Wo, bo, _run_kwargs=None):
    nc = _get_nc()
    in_maps = _make_in_maps(x, Wq, bq, Wk, bk, Wv, bv, Wo, bo)
    last_err = None
    for _attempt in range(3):
        try:
            res = run_bass_kernel_spmd(nc, in_maps, core_ids=list(range(NCORES)),
                                       **(_run_kwargs or {}))
            break
        except Exception as e:  # transient NRT/device hiccups: retry
            last_err = e
            import time as _time
            _time.sleep(2.0)
    else:
        raise last_err
    bo = np.asarray(bo, dtype=np.float32)
    out = np.empty((B, T, D), dtype=np.float32)
    for b in range(B):
        acc = res.results[4 * b]["outT"].astype(np.float32)
        for g in range(1, 4):
            acc += res.results[4 * b + g]["outT"].astype(np.float32)
        out[b] = acc.T + bo
    if _run_kwargs:
        _CACHE['last_results'] = res
    return out
